# revision 43
# baseline (speedup 1.0000x reference)
"""Sparse attention mixer (B=2,S=2048,D=1024,H=16,window=256 causal-banded)
on 8 trn2 NeuronCores.

Sharding: data-parallel over batch (2) x tensor-parallel over head groups (4).
Core c handles batch c//4, heads [4*(c%4), 4*(c%4)+4). Each core computes its
qkv projection slice, banded attention for its 4 heads, and a partial
out-projection over its 256 local dims; the host sums the 4 partials per batch
and adds the output bias.

Mask structure: mask[i,j] = 0 if j <= i+256 else -1e9  (causal OR |i-j|<=256,
clamped). Per 128-row query block qi, key blocks 0..qi+1 are fully allowed,
block qi+2 is lower-triangular (a<=b in transposed [sk,sq] layout), blocks
>qi+2 fully masked (skipped).

Schedule: PE-busy is the binding resource. The emission order front-loads the
minimal K/V/Q prefix for attention chunk 0, then drains a queue of deferred
PE work (remaining projections, prior chunk's out_proj) inside the attention
kb loops so the PE never waits on the scalar engine's exp stream.
"""

import sys
import types

import numpy as np

B, S, D, H = 2, 2048, 1024, 16
HD = 64          # head dim
HPC = 4          # heads per core
DL = HPC * HD    # 256 local dims per core
NCORES = 8
P = 128
NEG = np.float32(-1.0e9)
SCALE = float(HD) ** -0.5

# knobs for test harness
TRACE = False
TRACE_CORES = None
LAST_RESULTS = None

_MODULE_CACHE = {}


def _install_ntff_shim():
    """antenv.axon_hooks is absent in this image; register the NTFF profile
    hook via ctypes against the axon PJRT .so so trace=True works."""
    if 'antenv.axon_hooks' in sys.modules:
        return
    hook = None
    try:
        from trn_agent_boot.trn_boot import _ntff_profile_via_ctypes
        hook = _ntff_profile_via_ctypes('/opt/axon/libaxon_pjrt.so')
    except Exception:
        hook = None
    m = types.ModuleType('antenv.axon_hooks')
    m.get_axon_ntff_profile_hook = lambda: hook
    m.set_axon_ntff_profile_hook = lambda h: None
    sys.modules['antenv.axon_hooks'] = m


def _build_module():
    import concourse.mybir as mybir
    import concourse.tile as tile
    from concourse import bacc
    from concourse.bass import ts

    dt = mybir.dt
    f32 = dt.float32
    f32r = dt.float32r
    bf16 = dt.bfloat16
    AF = mybir.ActivationFunctionType

    ND = D // P      # 8 d-chunks
    NB = S // P      # 16 s-blocks of 128

    nc = bacc.Bacc('TRN2', target_bir_lowering=False, debug=False,
                   num_devices=NCORES)

    xT = nc.dram_tensor('xT', [D, S], bf16, kind='ExternalInput').ap()
    wqT = nc.dram_tensor('wqT', [D, DL], bf16, kind='ExternalInput').ap()
    wkT = nc.dram_tensor('wkT', [D, DL], bf16, kind='ExternalInput').ap()
    wvT = nc.dram_tensor('wvT', [D, DL], bf16, kind='ExternalInput').ap()
    woT = nc.dram_tensor('woT', [DL, D], bf16, kind='ExternalInput').ap()
    bq2 = nc.dram_tensor('bq2', [P, 2], f32, kind='ExternalInput').ap()
    bk2 = nc.dram_tensor('bk2', [P, 2], f32, kind='ExternalInput').ap()
    bvrow = nc.dram_tensor('bvrow', [1, DL], f32, kind='ExternalInput').ap()
    mask01 = nc.dram_tensor('mask01', [P, P], bf16, kind='ExternalInput').ap()
    onesr = nc.dram_tensor('onesr', [P, HD], f32r, kind='ExternalInput').ap()
    onescol = nc.dram_tensor('onescol', [P, 16, 1], bf16,
                             kind='ExternalInput').ap()
    out = nc.dram_tensor('out', [S, D], bf16, kind='ExternalOutput').ap()

    def r(ap):
        return ap

    def act_raw(out_ap, in_ap, func, scale=1.0):
        # raw InstActivation (out = func(scale*in)); bypasses dtype checks
        # for f32r outputs. ins = [in, bias, scale, alpha]
        eng = nc.scalar
        ins = [eng.lower_ap(in_ap),
               mybir.ImmediateValue(dtype=f32, value=0.0),
               mybir.ImmediateValue(dtype=f32, value=float(scale)),
               mybir.ImmediateValue(dtype=f32, value=0.0)]
        eng.add_instruction(mybir.InstActivation(
            name=nc.get_next_instruction_name(),
            func=func, ins=ins, outs=[eng.lower_ap(out_ap)]))

    with tile.TileContext(nc) as tc:
        with (
            tc.tile_pool(name='const', bufs=1) as cpool,
            tc.tile_pool(name='wp', bufs=1) as wpool,
            tc.tile_pool(name='persist', bufs=1) as ppool,
            tc.tile_pool(name='expp', bufs=6) as epool,
            tc.tile_pool(name='rp', bufs=2) as rpool,
            tc.tile_pool(name='ostage', bufs=3) as opool,
            tc.tile_pool(name='mm', bufs=2, space='PSUM') as mmp,
            tc.tile_pool(name='vps', bufs=2, space='PSUM') as vpsp,
            tc.tile_pool(name='avo', bufs=2, space='PSUM') as avop,
        ):
            # ---------------- input DMAs: wk + x first (kT proj is the
            # critical first PE work), spread over engine DGE queues --------
            xTv = xT.rearrange('(c p) s -> p c s', p=P)
            # PE warm-up: junk matmuls during the DMA preamble release the
            # HAM clock throttle before real work arrives (fp32 = multi-pass
            # on the PE, so 3 matmuls span ~2.5us)
            wmt = cpool.tile([P, 512], f32, name='wmt')
            nc.vector.memset(wmt[:], 1.0)
            wps = avop.tile([P, 512], f32, name='warm_ps', tag='avo')
            for i in range(3):
                nc.tensor.matmul(wps[:], r(wmt[:, 0:P]), r(wmt[:]),
                                 start=(i == 0), stop=(i == 2))
            wk_sb = wpool.tile([P, ND, DL], bf16, name='wk_sb')
            nc.scalar.dma_start(wk_sb[:, 0:4, :],
                                wkT.rearrange('(c p) o -> p c o', p=P)[:, 0:4])
            xt = ppool.tile([P, ND, S], bf16, name='xt')
            nc.sync.dma_start(xt[:, :, 0:512], xTv[:, :, 0:512])
            nc.gpsimd.dma_start(wk_sb[:, 4:8, :],
                                wkT.rearrange('(c p) o -> p c o', p=P)[:, 4:8])
            ones16_sb = cpool.tile([P, 16], bf16, name='ones16_sb')
            nc.sync.dma_start(ones16_sb[:],
                              onescol.rearrange('p n o -> p (n o)'))
            nc.sync.dma_start(xt[:, :, 512:1024], xTv[:, :, 512:1024])
            bq_sb = cpool.tile([P, 2], f32, name='bq_sb')
            nc.scalar.dma_start(bq_sb[:], bq2)
            # preload the one activation table that covers every func used
            # (exp, ln, copy) so no implicit table swaps are ever inserted
            try:
                from concourse.hw_specs import get_activation_tables
                _set_id = list(get_activation_tables(nc.m.arch)).index(
                    'natural_log_exp_and_others')
            except Exception:
                _set_id = 6
            nc.scalar.add_instruction(mybir.InstLoadActFuncSet(
                name=nc.get_next_instruction_name(),
                act_func_set_id=_set_id, ins=[], outs=[]))
            bk_sb = cpool.tile([P, 2], f32, name='bk_sb')
            nc.scalar.dma_start(bk_sb[:], bk2)
            bv_sb = cpool.tile([1, DL], f32, name='bv_sb')
            nc.scalar.dma_start(bv_sb[:], bvrow)
            wv_sb = wpool.tile([P, ND, DL], bf16, name='wv_sb')
            nc.scalar.dma_start(wv_sb[:], wvT.rearrange('(c p) o -> p c o', p=P))
            wq_sb = wpool.tile([P, ND, DL], bf16, name='wq_sb')
            nc.gpsimd.dma_start(wq_sb[:], wqT.rearrange('(c p) o -> p c o', p=P))
            nc.scalar.dma_start(xt[:, :, 1024:1536], xTv[:, :, 1024:1536])
            nc.scalar.dma_start(xt[:, :, 1536:2048], xTv[:, :, 1536:2048])
            m01_sb = cpool.tile([P, P], bf16, name='m01_sb')
            nc.gpsimd.dma_start(m01_sb[:], mask01)
            wo_sb = wpool.tile([P, 2, D], bf16, name='wo_sb')
            nc.gpsimd.dma_start(wo_sb[:], woT.rearrange('(t p) o -> p t o', p=P))
            # V bias broadcast to all partitions (added during V psum->sbuf)
            bvb_sb = cpool.tile([P, DL], f32, name='bvb_sb')
            nc.gpsimd.partition_broadcast(bvb_sb[:], bv_sb[:])
            onesr_sb = cpool.tile([P, HD], f32r, name='onesr_sb')
            nc.gpsimd.dma_start(onesr_sb[:], onesr)

            # ---------------- persistent intermediates ----------------
            # pair t holds heads {2t, 2t+1} stacked along partitions (64 each)
            qT_sb = [ppool.tile([P, S], bf16, name=f'qT{t}') for t in range(2)]
            kT_sb = [ppool.tile([P, S], bf16, name=f'kT{t}') for t in range(2)]
            # V blocks: per s-block, per head: 64 V columns + 1 ones column
            v_sb = ppool.tile([P, NB, HPC * (HD + 1)], bf16, name='v_sb')
            # attn outT pairs: partitions = 128 local dims of pair t, free = s
            aoT_sb = [ppool.tile([P, S], bf16, name=f'aoT{t}') for t in range(2)]

            # ---------------- projection group emitters ----------------
            # qT/kT: per (q/k, pair t, s-range): psum [128, w], 8 c-chunk
            # accumulation MMs (moving = x, up to 1024 wide bf16), then
            # scale+bias on the psum->sbuf move.
            def emit_qk(which, t, s0, w):
                wsb, bsb, dst, scale = (
                    (wq_sb, bq_sb, qT_sb, SCALE) if which == 'q' else
                    (wk_sb, bk_sb, kT_sb, 1.0))
                ps = mmp.tile([P, 512], f32, name=f'{which}ps{t}_{s0}',
                              tag='mm')
                for c in range(ND):
                    nc.tensor.matmul(
                        ps[:, 0:w], r(wsb[:, c, ts(t, P)]),
                        r(xt[:, c, s0:s0 + w]),
                        start=(c == 0), stop=(c == ND - 1))
                nc.vector.tensor_scalar(
                    out=dst[t][:, s0:s0 + w], in0=ps[:, 0:w],
                    scalar1=scale, scalar2=bsb[:, t:t + 1],
                    op0=mybir.AluOpType.mult,
                    op1=mybir.AluOpType.add)

            # V: out [128(s), 256(o)] per s-block; bias added on the
            # psum->sbuf move as one strided add (dst skips ones columns)
            def emit_v(sb):
                vps = vpsp.tile([P, DL], f32, name=f'v_ps{sb}', tag='vps')
                for c in range(ND):
                    nc.tensor.matmul(
                        vps[:], r(xt[:, c, ts(sb, P)]), r(wv_sb[:, c, :]),
                        start=(c == 0), stop=(c == ND - 1))
                dst = v_sb[:, sb, :].rearrange('p (h e) -> p h e', e=HD + 1)
                nc.vector.tensor_add(
                    dst[:, :, 0:HD],
                    vps.rearrange('p (h e) -> p h e', e=HD),
                    bvb_sb.rearrange('p (h e) -> p h e', e=HD))

            # out_proj for s-block m, d-half n: psum [128, 512], 2 MMs; the
            # psum->sbuf stage runs on ACT (chunks 0,1,3) or DVE (chunk 2)
            def emit_oproj(m, n, stage_eng):
                ops = vpsp.tile([P, 512], f32, name=f'o_ps{m}_{n}', tag='vps')
                for t in range(2):
                    nc.tensor.matmul(ops[:],
                                     r(aoT_sb[t][:, ts(m, P)]),
                                     r(wo_sb[:, t, ts(n, 512)]),
                                     start=(t == 0), stop=(t == 1))
                ost = opool.tile([P, 512], bf16, name=f'ost{m}_{n}', tag='ost')
                if stage_eng == 'act':
                    nc.scalar.activation(ost[:], ops[:], AF.Copy)
                else:
                    nc.vector.tensor_copy(ost[:], ops[:])
                nc.sync.dma_start(out[ts(m, P), ts(n, 512)], ost[:])

            # ---------------- deferred-work (filler) queue ----------------
            fillers = []

            def drain(n=1):
                for _ in range(n):
                    if fillers:
                        fillers.pop(0)()

            # ---------------- attention chunk ----------------
            def norm_heads(c, t_hi_list, gr, aou, mode):
                """1/d rows live in gr at partitions 32*(2t+hi) (f32r for
                'pe', f32 for 'gps'); broadcast to 64 partitions and
                multiply into aoT."""
                for t, hi in t_hi_list:
                    row = 32 * (2 * t + hi) if len(t_hi_list) > 2 else 32 * hi
                    if mode == 'pe':
                        rp = vpsp.tile([HD, 512], f32, name=f'rb{c}_{t}{hi}',
                                       tag='vps')
                        nc.tensor.matmul(rp[:], onesr_sb[row:row + 1, :],
                                         gr[row:row + 1, :],
                                         start=True, stop=True,
                                         tile_position=(row, 0))
                    else:
                        rrow = rpool.tile([1, 512], f32, name=f'rr{c}_{t}{hi}',
                                          tag='rr', bufs=2)
                        nc.vector.tensor_copy(rrow[:], gr[row:row + 1, :])
                        rp = rpool.tile([HD, 512], f32, name=f'rb{c}_{t}{hi}',
                                        tag='rb', bufs=2)
                        nc.gpsimd.partition_broadcast(rp[:], rrow[:],
                                                      channels=HD)
                    nc.vector.tensor_mul(
                        aoT_sb[t][64 * hi:64 * hi + 64, ts(c, 512)],
                        aou[2 * t + hi][0:HD, :], rp[:])

            def emit_att(c, fill_at, norm='chunk_gps'):
                """Attention for query chunk c (s columns [512c, 512c+512)).
                fill_at[t] = set of kb indices after whose scores+AV emission
                one filler group is drained. norm: 'chunk_gps' / 'chunk_pe'
                (one Ln/Exp per chunk; broadcast on gpsimd or PE) or
                'pair_pe' (per head-pair, for the final chunk's tail)."""
                aou = []                # unnormalized [attn@V; sums] per head
                gdt = f32 if norm == 'chunk_gps' else f32r
                if norm.startswith('chunk'):
                    g = rpool.tile([97, 512], f32, name=f'g{c}', tag='g',
                                   bufs=2)
                    nc.vector.memset(g[:], 1.0)
                for t in range(2):      # head pair; heads 2t (rows 0:64), 2t+1
                    kb_max = min(NB, 4 * c + 6)   # key blocks 0..kb_max-1
                    avo = [avop.tile([HD + 1, 512], f32,
                                     name=f'avo{c}_{2 * t + hi}', tag='avo')
                           for hi in range(2)]
                    # software-pipelined: scores(kb) pair emitted back-to-back
                    # into one 2-bank psum tile (halves: hi=0 -> [0:512],
                    # hi=1 -> [512:1024]; distinct PE row groups overlap),
                    # one merged exp per kb, AV(kb-1) after scores(kb)
                    def emit_av(pend, last):
                        pet, pn0, pkb = pend
                        for hi in range(2):
                            h = 2 * t + hi
                            nc.tensor.matmul(
                                avo[hi][:, pn0:],
                                r(v_sb[:, pkb,
                                       h * (HD + 1):(h + 1) * (HD + 1)]),
                                r(pet[:, 512 * hi + pn0:512 * (hi + 1)]),
                                start=(pkb == 0), stop=last,
                                skip_group_check=True)

                    pend = None
                    for kb in range(kb_max):
                        z = max(0, kb - 4 * c - 2)   # fully-masked sub-blocks
                        n0 = P * z
                        lb = kb - 2 - 4 * c          # banded sub-block index
                        sps = mmp.tile([P, 1024], f32,
                                       name=f's_ps{c}_{t}_{kb}', tag='mm')
                        for hi in range(2):
                            nc.tensor.matmul(
                                sps[:, 512 * hi + n0:512 * (hi + 1)],
                                r(kT_sb[t][64 * hi:64 * hi + 64, ts(kb, P)]),
                                r(qT_sb[t][64 * hi:64 * hi + 64,
                                           512 * c + n0:512 * (c + 1)]),
                                start=True, stop=True)
                        et = epool.tile([P, 1024], bf16,
                                        name=f'exp{c}_{t}_{kb}', tag='exp')
                        spsv = sps.rearrange('p (u q) -> p u q', u=2)
                        etv = et.rearrange('p (u q) -> p u q', u=2)
                        nc.scalar.activation(etv[:, :, n0:], spsv[:, :, n0:],
                                             AF.Exp)
                        if 0 <= lb < 4:
                            nc.vector.tensor_mul(
                                etv[:, :, 128 * lb:128 * lb + 128],
                                etv[:, :, 128 * lb:128 * lb + 128],
                                m01_sb[:, None, :].broadcast_to([P, 2, P]))
                        if pend is not None:
                            emit_av(pend, False)
                        pend = (et, n0, kb)
                        if kb in fill_at[t]:
                            drain()
                    emit_av(pend, True)
                    # gather the sums rows straight from psum BEFORE the big
                    # staging copies, so 1/d = exp(-ln d) overlaps them
                    if norm == 'pair_pe':
                        g = rpool.tile([33, 512], f32, name=f'g{c}_{t}',
                                       tag='g', bufs=2)
                        nc.vector.memset(g[:], 1.0)
                    for hi in range(2):
                        row = 32 * (2 * t + hi) if norm != 'pair_pe' else 32 * hi
                        nc.vector.tensor_copy(g[row:row + 1, :],
                                              avo[hi][64:65, :])
                    # release avo: stage unnormalized result to SBUF
                    for hi in range(2):
                        ao = rpool.tile([HD + 1, 512], f32,
                                        name=f'aou{c}_{2 * t + hi}', tag='aou',
                                        bufs=8)
                        nc.vector.tensor_copy(ao[:], avo[hi][:])
                        aou.append(ao)
                    if norm == 'pair_pe':
                        gl = rpool.tile([33, 512], f32, name=f'gl{c}_{t}',
                                        tag='gl', bufs=2)
                        act_raw(gl[:], g[:], AF.Ln)
                        gr = rpool.tile([33, 512], f32r, name=f'gr{c}_{t}',
                                        tag='gr', bufs=2)
                        act_raw(gr[:], gl[:], AF.Exp, scale=-1.0)
                        norm_heads(c, [(t, 0), (t, 1)], gr, aou, 'pe')
                if norm.startswith('chunk'):
                    gl = rpool.tile([97, 512], f32, name=f'gl{c}', tag='gl',
                                    bufs=2)
                    act_raw(gl[:], g[:], AF.Ln)
                    gr = rpool.tile([97, 512], gdt, name=f'gr{c}', tag='gr',
                                    bufs=2)
                    act_raw(gr[:], gl[:], AF.Exp, scale=-1.0)
                    norm_heads(c, [(t, hi) for t in range(2)
                                   for hi in range(2)], gr, aou,
                               'pe' if norm == 'chunk_pe' else 'gps')

            # ---------------- emission schedule ----------------
            # prefix: only what attention chunk 0's first 4 key blocks need,
            # all derivable from the first x s-quarter; the q2-dependent
            # groups drain as att(0) fillers so the PE never waits on x
            emit_qk('k', 0, 0, 512)
            emit_qk('k', 1, 0, 512)
            emit_v(0)
            emit_v(1)
            emit_qk('q', 0, 0, 512)
            emit_qk('q', 1, 0, 512)
            emit_v(2)
            emit_v(3)
            # per-head ones columns of v_sb (strided DVE copies; memset
            # cannot write bf16 and elementwise DMAs are pathological).
            # Emitted after the prefix so they don't head-block the DVE
            # queue while the ones DMA is in flight.
            for h in range(HPC):
                c0 = h * (HD + 1) + HD
                nc.vector.tensor_copy(
                    v_sb[:, :, c0:c0 + 1],
                    ones16_sb.rearrange('p (n o) -> p n o', o=1))

            # deferred projection work, drained inside attention loops.
            # att(0): s[512:1024] projections (kT blocks 4,5 / V 4,5 feed
            # its own tail kbs), then V 6..9 and kT 8..11 for att(1).
            fillers += [lambda: emit_qk('k', 0, 512, 512),
                        lambda: emit_qk('k', 1, 512, 512),
                        lambda: emit_qk('q', 0, 512, 512),
                        lambda: emit_qk('q', 1, 512, 512),
                        lambda: emit_v(4), lambda: emit_v(5),
                        lambda: emit_v(6), lambda: emit_v(7),
                        lambda: emit_qk('k', 0, 1024, 512),
                        lambda: emit_qk('k', 1, 1024, 512),
                        lambda: emit_v(8), lambda: emit_v(9)]
            emit_att(0, fill_at=[{0, 1, 2, 3, 4, 5}, {0, 1, 2, 3, 4, 5}],
                     norm='chunk_gps')

            # att(1): fill with qT chunk-1, V 10..13 (for att(2))
            fillers += [lambda: emit_qk('q', 0, 1024, 512),
                        lambda: emit_qk('q', 0, 1536, 512),
                        lambda: emit_qk('q', 1, 1024, 512),
                        lambda: emit_qk('q', 1, 1536, 512),
                        lambda: emit_v(10), lambda: emit_v(11),
                        lambda: emit_v(12), lambda: emit_v(13)]
            emit_att(1, fill_at=[{0, 2, 4, 6}, {0, 2, 4, 6}],
                     norm='chunk_gps')

            # kT tail blocks 12..15 + out_proj(0) + V 14,15 during att(2)
            fillers += [lambda: emit_qk('k', 0, 1536, 512),
                        lambda: emit_qk('k', 1, 1536, 512)]
            for m in range(4):
                for n in range(2):
                    fillers.append(
                        lambda m=m, n=n: emit_oproj(m, n, 'dve'))
            fillers += [lambda: emit_v(14), lambda: emit_v(15)]
            emit_att(2, fill_at=[{0, 2, 4, 6, 8, 10},
                                 {0, 2, 4, 6, 8, 10}], norm='chunk_pe')

            # out_proj(1) and out_proj(2) during att(3)
            for m in range(4, 12):
                for n in range(2):
                    fillers.append(
                        lambda m=m, n=n: emit_oproj(m, n, 'dve'))
            emit_att(3, fill_at=[{0, 2, 4, 6, 8, 10, 12, 14},
                                 {0, 2, 4, 6, 8, 10, 12, 14}],
                     norm='pair_pe')
            drain(len(fillers))

            # tail: out_proj(3); alternate staging engines (both idle now)
            for i, (m, n) in enumerate(
                    [(m, n) for m in range(12, 16) for n in range(2)]):
                emit_oproj(m, n, 'act' if i % 2 == 0 else 'dve')

    nc.compile()
    return nc


def _get_module():
    if 'nc' not in _MODULE_CACHE:
        _MODULE_CACHE['nc'] = _build_module()
    return _MODULE_CACHE['nc']


def _make_in_maps(x, in_proj_w, in_proj_b, out_proj_w):
    import ml_dtypes
    bf = ml_dtypes.bfloat16
    x = np.asarray(x, np.float32)
    in_proj_w = np.asarray(in_proj_w, np.float32)
    in_proj_b = np.asarray(in_proj_b, np.float32)
    out_proj_w = np.asarray(out_proj_w, np.float32)

    mask01b = (np.arange(P)[:, None] <= np.arange(P)[None, :])

    xT = [np.ascontiguousarray(x[b].T) for b in range(B)]
    in_maps = []
    for core in range(NCORES):
        b, hg = core // 4, core % 4
        sl = slice(DL * hg, DL * hg + DL)
        wq = in_proj_w[0 * D:1 * D][sl]
        wk = in_proj_w[1 * D:2 * D][sl]
        wv = in_proj_w[2 * D:3 * D][sl]
        bq = in_proj_b[0 * D:1 * D][sl]
        bk = in_proj_b[1 * D:2 * D][sl]
        bv = in_proj_b[2 * D:3 * D][sl]
        in_maps.append({
            'xT': xT[b].astype(bf),
            'wqT': np.ascontiguousarray(wq.T).astype(bf),
            'wkT': np.ascontiguousarray(wk.T).astype(bf),
            'wvT': np.ascontiguousarray(wv.T).astype(bf),
            'woT': np.ascontiguousarray(out_proj_w[:, sl].T).astype(bf),
            'bq2': np.ascontiguousarray((bq * SCALE).reshape(2, P).T),
            'bk2': np.ascontiguousarray(bk.reshape(2, P).T),
            'bvrow': bv.reshape(1, DL).copy(),
            'mask01': mask01b.astype(ml_dtypes.bfloat16),
            'onesr': np.ones((P, HD), np.float32),
            'onescol': np.ones((P, 16, 1), ml_dtypes.bfloat16),
        })
    return in_maps


def kernel(x, in_proj_w, in_proj_b, out_proj_w, out_proj_b):
    global LAST_RESULTS
    _install_ntff_shim()
    from concourse import bass_utils

    nc = _get_module()
    in_maps = _make_in_maps(x, in_proj_w, in_proj_b, out_proj_w)
    res = bass_utils.run_bass_kernel_spmd(
        nc, in_maps, core_ids=list(range(NCORES)),
        trace=TRACE,
        **({'trace_cores': TRACE_CORES} if TRACE_CORES else {}))
    LAST_RESULTS = res

    out = np.zeros((B, S, D), np.float32)
    for core in range(NCORES):
        out[core // 4] += np.asarray(res.results[core]['out'], np.float32)
    out += np.asarray(out_proj_b, np.float32)
    return out


# revision 46
# speedup vs baseline: 1.2650x; 1.2650x over previous
"""Sparse attention mixer (B=2,S=2048,D=1024,H=16,window=256 causal-banded)
on 8 trn2 NeuronCores.

Sharding: data-parallel over batch (2) x tensor-parallel over head groups (4).
Core c handles batch c//4, heads [4*(c%4), 4*(c%4)+4). Each core computes its
qkv projection slice, banded attention for its 4 heads, and a partial
out-projection over its 256 local dims; the host sums the 4 partials per batch
and adds the output bias.

Mask structure: mask[i,j] = 0 if j <= i+256 else -1e9  (causal OR |i-j|<=256,
clamped). Per 128-row query block qi, key blocks 0..qi+1 are fully allowed,
block qi+2 is lower-triangular (a<=b in transposed [sk,sq] layout), blocks
>qi+2 fully masked (skipped).

Schedule: PE-busy is the binding resource. The emission order front-loads the
minimal K/V/Q prefix for attention chunk 0, then drains a queue of deferred
PE work (remaining projections, prior chunk's out_proj) inside the attention
kb loops so the PE never waits on the scalar engine's exp stream.
"""

import sys
import types

import numpy as np

B, S, D, H = 2, 2048, 1024, 16
HD = 64          # head dim
HPC = 4          # heads per core
DL = HPC * HD    # 256 local dims per core
NCORES = 8
P = 128
NEG = np.float32(-1.0e9)
SCALE = float(HD) ** -0.5

# knobs for test harness
TRACE = False
TRACE_CORES = None
LAST_RESULTS = None

_MODULE_CACHE = {}


def _install_ntff_shim():
    """antenv.axon_hooks is absent in this image; register the NTFF profile
    hook via ctypes against the axon PJRT .so so trace=True works."""
    if 'antenv.axon_hooks' in sys.modules:
        return
    hook = None
    try:
        from trn_agent_boot.trn_boot import _ntff_profile_via_ctypes
        hook = _ntff_profile_via_ctypes('/opt/axon/libaxon_pjrt.so')
    except Exception:
        hook = None
    m = types.ModuleType('antenv.axon_hooks')
    m.get_axon_ntff_profile_hook = lambda: hook
    m.set_axon_ntff_profile_hook = lambda h: None
    sys.modules['antenv.axon_hooks'] = m


def _build_module():
    import concourse.mybir as mybir
    import concourse.tile as tile
    from concourse import bacc
    from concourse.bass import ts

    dt = mybir.dt
    f32 = dt.float32
    f32r = dt.float32r
    bf16 = dt.bfloat16
    AF = mybir.ActivationFunctionType

    ND = D // P      # 8 d-chunks
    NB = S // P      # 16 s-blocks of 128

    nc = bacc.Bacc('TRN2', target_bir_lowering=False, debug=False,
                   num_devices=NCORES)

    xT = nc.dram_tensor('xT', [D, S], bf16, kind='ExternalInput').ap()
    wqT = nc.dram_tensor('wqT', [D, DL], bf16, kind='ExternalInput').ap()
    wkT = nc.dram_tensor('wkT', [D, DL], bf16, kind='ExternalInput').ap()
    wvT = nc.dram_tensor('wvT', [D, DL], bf16, kind='ExternalInput').ap()
    woT = nc.dram_tensor('woT', [DL, D], bf16, kind='ExternalInput').ap()
    bq2 = nc.dram_tensor('bq2', [P, 2], f32, kind='ExternalInput').ap()
    bk2 = nc.dram_tensor('bk2', [P, 2], f32, kind='ExternalInput').ap()
    bvrow = nc.dram_tensor('bvrow', [1, DL], f32, kind='ExternalInput').ap()
    mask01 = nc.dram_tensor('mask01', [P, P], bf16, kind='ExternalInput').ap()
    onesr = nc.dram_tensor('onesr', [P, HD], f32r, kind='ExternalInput').ap()
    onescol = nc.dram_tensor('onescol', [P, 16, 1], bf16,
                             kind='ExternalInput').ap()
    out = nc.dram_tensor('out', [S, D], f32, kind='ExternalOutput').ap()

    def r(ap):
        return ap

    def act_raw(out_ap, in_ap, func, scale=1.0):
        # raw InstActivation (out = func(scale*in)); bypasses dtype checks
        # for f32r outputs. ins = [in, bias, scale, alpha]
        eng = nc.scalar
        ins = [eng.lower_ap(in_ap),
               mybir.ImmediateValue(dtype=f32, value=0.0),
               mybir.ImmediateValue(dtype=f32, value=float(scale)),
               mybir.ImmediateValue(dtype=f32, value=0.0)]
        eng.add_instruction(mybir.InstActivation(
            name=nc.get_next_instruction_name(),
            func=func, ins=ins, outs=[eng.lower_ap(out_ap)]))

    with tile.TileContext(nc) as tc:
        with (
            tc.tile_pool(name='const', bufs=1) as cpool,
            tc.tile_pool(name='wp', bufs=1) as wpool,
            tc.tile_pool(name='persist', bufs=1) as ppool,
            tc.tile_pool(name='expp', bufs=6) as epool,
            tc.tile_pool(name='rp', bufs=2) as rpool,
            tc.tile_pool(name='ostage', bufs=3) as opool,
            tc.tile_pool(name='mm', bufs=2, space='PSUM') as mmp,
            tc.tile_pool(name='vps', bufs=2, space='PSUM') as vpsp,
            tc.tile_pool(name='avo', bufs=2, space='PSUM') as avop,
        ):
            # ---------------- input DMAs: wk + x first (kT proj is the
            # critical first PE work), spread over engine DGE queues --------
            xTv = xT.rearrange('(c p) s -> p c s', p=P)
            # PE warm-up: junk matmuls during the DMA preamble release the
            # HAM clock throttle before real work arrives (fp32 = multi-pass
            # on the PE, so 3 matmuls span ~2.5us)
            wmt = cpool.tile([P, 512], f32, name='wmt')
            nc.vector.memset(wmt[:], 1.0)
            wps = avop.tile([P, 512], f32, name='warm_ps', tag='avo')
            for i in range(3):
                nc.tensor.matmul(wps[:], r(wmt[:, 0:P]), r(wmt[:]),
                                 start=(i == 0), stop=(i == 2))
            wk_sb = wpool.tile([P, ND, DL], bf16, name='wk_sb')
            nc.scalar.dma_start(wk_sb[:, 0:4, :],
                                wkT.rearrange('(c p) o -> p c o', p=P)[:, 0:4])
            xt = ppool.tile([P, ND, S], bf16, name='xt')
            nc.sync.dma_start(xt[:, :, 0:512], xTv[:, :, 0:512])
            nc.gpsimd.dma_start(wk_sb[:, 4:8, :],
                                wkT.rearrange('(c p) o -> p c o', p=P)[:, 4:8])
            ones16_sb = cpool.tile([P, 16], bf16, name='ones16_sb')
            nc.sync.dma_start(ones16_sb[:],
                              onescol.rearrange('p n o -> p (n o)'))
            nc.sync.dma_start(xt[:, :, 512:1024], xTv[:, :, 512:1024])
            bq_sb = cpool.tile([P, 2], f32, name='bq_sb')
            nc.scalar.dma_start(bq_sb[:], bq2)
            # preload the one activation table that covers every func used
            # (exp, ln, copy) so no implicit table swaps are ever inserted
            try:
                from concourse.hw_specs import get_activation_tables
                _set_id = list(get_activation_tables(nc.m.arch)).index(
                    'natural_log_exp_and_others')
            except Exception:
                _set_id = 6
            nc.scalar.add_instruction(mybir.InstLoadActFuncSet(
                name=nc.get_next_instruction_name(),
                act_func_set_id=_set_id, ins=[], outs=[]))
            bk_sb = cpool.tile([P, 2], f32, name='bk_sb')
            nc.scalar.dma_start(bk_sb[:], bk2)
            bv_sb = cpool.tile([1, DL], f32, name='bv_sb')
            nc.scalar.dma_start(bv_sb[:], bvrow)
            wv_sb = wpool.tile([P, ND, DL], bf16, name='wv_sb')
            nc.scalar.dma_start(wv_sb[:], wvT.rearrange('(c p) o -> p c o', p=P))
            wq_sb = wpool.tile([P, ND, DL], bf16, name='wq_sb')
            nc.gpsimd.dma_start(wq_sb[:], wqT.rearrange('(c p) o -> p c o', p=P))
            nc.scalar.dma_start(xt[:, :, 1024:1536], xTv[:, :, 1024:1536])
            nc.scalar.dma_start(xt[:, :, 1536:2048], xTv[:, :, 1536:2048])
            m01_sb = cpool.tile([P, P], bf16, name='m01_sb')
            nc.gpsimd.dma_start(m01_sb[:], mask01)
            wo_sb = wpool.tile([P, 2, D], bf16, name='wo_sb')
            nc.gpsimd.dma_start(wo_sb[:], woT.rearrange('(t p) o -> p t o', p=P))
            # V bias broadcast to all partitions (added during V psum->sbuf)
            bvb_sb = cpool.tile([P, DL], f32, name='bvb_sb')
            nc.gpsimd.partition_broadcast(bvb_sb[:], bv_sb[:])
            onesr_sb = cpool.tile([P, HD], f32r, name='onesr_sb')
            nc.gpsimd.dma_start(onesr_sb[:], onesr)

            # ---------------- persistent intermediates ----------------
            # pair t holds heads {2t, 2t+1} stacked along partitions (64 each)
            qT_sb = [ppool.tile([P, S], bf16, name=f'qT{t}') for t in range(2)]
            kT_sb = [ppool.tile([P, S], bf16, name=f'kT{t}') for t in range(2)]
            # V blocks: per s-block, per head: 64 V columns + 1 ones column
            v_sb = ppool.tile([P, NB, HPC * (HD + 1)], bf16, name='v_sb')
            # attn outT pairs: partitions = 128 local dims of pair t, free = s
            aoT_sb = [ppool.tile([P, S], bf16, name=f'aoT{t}') for t in range(2)]

            # ---------------- projection group emitters ----------------
            # qT/kT: per (q/k, pair t, s-range): psum [128, w], 8 c-chunk
            # accumulation MMs (moving = x, up to 1024 wide bf16), then
            # scale+bias on the psum->sbuf move.
            def emit_qk(which, t, s0, w):
                wsb, bsb, dst, scale = (
                    (wq_sb, bq_sb, qT_sb, SCALE) if which == 'q' else
                    (wk_sb, bk_sb, kT_sb, 1.0))
                ps = mmp.tile([P, 512], f32, name=f'{which}ps{t}_{s0}',
                              tag='mm')
                for c in range(ND):
                    nc.tensor.matmul(
                        ps[:, 0:w], r(wsb[:, c, ts(t, P)]),
                        r(xt[:, c, s0:s0 + w]),
                        start=(c == 0), stop=(c == ND - 1))
                nc.vector.tensor_scalar(
                    out=dst[t][:, s0:s0 + w], in0=ps[:, 0:w],
                    scalar1=scale, scalar2=bsb[:, t:t + 1],
                    op0=mybir.AluOpType.mult,
                    op1=mybir.AluOpType.add)

            # V: out [128(s), 256(o)] per s-block; bias added on the
            # psum->sbuf move as one strided add (dst skips ones columns)
            def emit_v(sb):
                vps = vpsp.tile([P, DL], f32, name=f'v_ps{sb}', tag='vps')
                for c in range(ND):
                    nc.tensor.matmul(
                        vps[:], r(xt[:, c, ts(sb, P)]), r(wv_sb[:, c, :]),
                        start=(c == 0), stop=(c == ND - 1))
                dst = v_sb[:, sb, :].rearrange('p (h e) -> p h e', e=HD + 1)
                nc.vector.tensor_add(
                    dst[:, :, 0:HD],
                    vps.rearrange('p (h e) -> p h e', e=HD),
                    bvb_sb.rearrange('p (h e) -> p h e', e=HD))

            # out_proj for s-block m, d-half n: psum [128, 512], 2 MMs; the
            # psum->sbuf stage runs on ACT (chunks 0,1,3) or DVE (chunk 2)
            def emit_oproj(m, n, stage_eng):
                ops = vpsp.tile([P, 512], f32, name=f'o_ps{m}_{n}', tag='vps')
                for t in range(2):
                    nc.tensor.matmul(ops[:],
                                     r(aoT_sb[t][:, ts(m, P)]),
                                     r(wo_sb[:, t, ts(n, 512)]),
                                     start=(t == 0), stop=(t == 1))
                ost = opool.tile([P, 512], f32, name=f'ost{m}_{n}', tag='ost')
                if stage_eng == 'act':
                    nc.scalar.activation(ost[:], ops[:], AF.Copy)
                else:
                    nc.vector.tensor_copy(ost[:], ops[:])
                nc.sync.dma_start(out[ts(m, P), ts(n, 512)], ost[:])

            # ---------------- deferred-work (filler) queue ----------------
            fillers = []

            def drain(n=1):
                for _ in range(n):
                    if fillers:
                        fillers.pop(0)()

            # ---------------- attention chunk ----------------
            def norm_heads(c, t_hi_list, gr, aou, mode):
                """1/d rows live in gr at partitions 32*(2t+hi) (f32r for
                'pe', f32 for 'gps'); broadcast to 64 partitions and
                multiply into aoT."""
                for t, hi in t_hi_list:
                    row = 32 * (2 * t + hi) if len(t_hi_list) > 2 else 32 * hi
                    if mode == 'pe':
                        rp = vpsp.tile([HD, 512], f32, name=f'rb{c}_{t}{hi}',
                                       tag='vps')
                        nc.tensor.matmul(rp[:], onesr_sb[row:row + 1, :],
                                         gr[row:row + 1, :],
                                         start=True, stop=True,
                                         tile_position=(row, 0))
                    else:
                        rrow = rpool.tile([1, 512], f32, name=f'rr{c}_{t}{hi}',
                                          tag='rr', bufs=2)
                        nc.vector.tensor_copy(rrow[:], gr[row:row + 1, :])
                        rp = rpool.tile([HD, 512], f32, name=f'rb{c}_{t}{hi}',
                                        tag='rb', bufs=2)
                        nc.gpsimd.partition_broadcast(rp[:], rrow[:],
                                                      channels=HD)
                    nc.vector.tensor_mul(
                        aoT_sb[t][64 * hi:64 * hi + 64, ts(c, 512)],
                        aou[2 * t + hi][0:HD, :], rp[:])

            def emit_att(c, fill_at, norm='chunk_gps'):
                """Attention for query chunk c (s columns [512c, 512c+512)).
                fill_at[t] = set of kb indices after whose scores+AV emission
                one filler group is drained. norm: 'chunk_gps' / 'chunk_pe'
                (one Ln/Exp per chunk; broadcast on gpsimd or PE) or
                'pair_pe' (per head-pair, for the final chunk's tail)."""
                aou = []                # unnormalized [attn@V; sums] per head
                gdt = f32 if norm == 'chunk_gps' else f32r
                if norm.startswith('chunk'):
                    g = rpool.tile([97, 512], f32, name=f'g{c}', tag='g',
                                   bufs=2)
                    nc.vector.memset(g[:], 1.0)
                for t in range(2):      # head pair; heads 2t (rows 0:64), 2t+1
                    kb_max = min(NB, 4 * c + 6)   # key blocks 0..kb_max-1
                    avo = [avop.tile([HD + 1, 512], f32,
                                     name=f'avo{c}_{2 * t + hi}', tag='avo')
                           for hi in range(2)]
                    # software-pipelined: scores(kb) pair emitted back-to-back
                    # into one 2-bank psum tile (halves: hi=0 -> [0:512],
                    # hi=1 -> [512:1024]; distinct PE row groups overlap),
                    # one merged exp per kb, AV(kb-1) after scores(kb)
                    def emit_av(pend, last):
                        pet, pn0, pkb = pend
                        for hi in range(2):
                            h = 2 * t + hi
                            nc.tensor.matmul(
                                avo[hi][:, pn0:],
                                r(v_sb[:, pkb,
                                       h * (HD + 1):(h + 1) * (HD + 1)]),
                                r(pet[:, 512 * hi + pn0:512 * (hi + 1)]),
                                start=(pkb == 0), stop=last,
                                skip_group_check=True)

                    pend = None
                    for kb in range(kb_max):
                        z = max(0, kb - 4 * c - 2)   # fully-masked sub-blocks
                        n0 = P * z
                        lb = kb - 2 - 4 * c          # banded sub-block index
                        sps = mmp.tile([P, 1024], f32,
                                       name=f's_ps{c}_{t}_{kb}', tag='mm')
                        for hi in range(2):
                            nc.tensor.matmul(
                                sps[:, 512 * hi + n0:512 * (hi + 1)],
                                r(kT_sb[t][64 * hi:64 * hi + 64, ts(kb, P)]),
                                r(qT_sb[t][64 * hi:64 * hi + 64,
                                           512 * c + n0:512 * (c + 1)]),
                                start=True, stop=True)
                        et = epool.tile([P, 1024], bf16,
                                        name=f'exp{c}_{t}_{kb}', tag='exp')
                        spsv = sps.rearrange('p (u q) -> p u q', u=2)
                        etv = et.rearrange('p (u q) -> p u q', u=2)
                        nc.scalar.activation(etv[:, :, n0:], spsv[:, :, n0:],
                                             AF.Exp)
                        if 0 <= lb < 4:
                            nc.vector.tensor_mul(
                                etv[:, :, 128 * lb:128 * lb + 128],
                                etv[:, :, 128 * lb:128 * lb + 128],
                                m01_sb[:, None, :].broadcast_to([P, 2, P]))
                        if pend is not None:
                            emit_av(pend, False)
                        pend = (et, n0, kb)
                        if kb in fill_at[t]:
                            drain()
                    emit_av(pend, True)
                    # gather the sums rows straight from psum BEFORE the big
                    # staging copies, so 1/d = exp(-ln d) overlaps them
                    if norm == 'pair_pe':
                        g = rpool.tile([33, 512], f32, name=f'g{c}_{t}',
                                       tag='g', bufs=2)
                        nc.vector.memset(g[:], 1.0)
                    for hi in range(2):
                        row = 32 * (2 * t + hi) if norm != 'pair_pe' else 32 * hi
                        nc.vector.tensor_copy(g[row:row + 1, :],
                                              avo[hi][64:65, :])
                    # release avo: stage unnormalized result to SBUF
                    for hi in range(2):
                        ao = rpool.tile([HD + 1, 512], f32,
                                        name=f'aou{c}_{2 * t + hi}', tag='aou',
                                        bufs=8)
                        nc.vector.tensor_copy(ao[:], avo[hi][:])
                        aou.append(ao)
                    if norm == 'pair_pe':
                        gl = rpool.tile([33, 512], f32, name=f'gl{c}_{t}',
                                        tag='gl', bufs=2)
                        act_raw(gl[:], g[:], AF.Ln)
                        gr = rpool.tile([33, 512], f32r, name=f'gr{c}_{t}',
                                        tag='gr', bufs=2)
                        act_raw(gr[:], gl[:], AF.Exp, scale=-1.0)
                        norm_heads(c, [(t, 0), (t, 1)], gr, aou, 'pe')
                if norm.startswith('chunk'):
                    gl = rpool.tile([97, 512], f32, name=f'gl{c}', tag='gl',
                                    bufs=2)
                    act_raw(gl[:], g[:], AF.Ln)
                    gr = rpool.tile([97, 512], gdt, name=f'gr{c}', tag='gr',
                                    bufs=2)
                    act_raw(gr[:], gl[:], AF.Exp, scale=-1.0)
                    norm_heads(c, [(t, hi) for t in range(2)
                                   for hi in range(2)], gr, aou,
                               'pe' if norm == 'chunk_pe' else 'gps')

            # ---------------- emission schedule ----------------
            # prefix: everything attention chunk 0 needs
            emit_qk('k', 0, 0, 512)
            emit_qk('k', 1, 0, 512)
            emit_qk('k', 0, 512, 512)
            emit_qk('k', 1, 512, 512)
            emit_v(0)
            emit_v(1)
            emit_qk('q', 0, 0, 512)
            emit_qk('q', 1, 0, 512)
            emit_qk('q', 0, 512, 512)
            emit_qk('q', 1, 512, 512)
            for sb in (2, 3, 4, 5):
                emit_v(sb)
            # per-head ones columns of v_sb (strided DVE copies; memset
            # cannot write bf16 and elementwise DMAs are pathological).
            # Emitted after the prefix so they don't head-block the DVE
            # queue while the ones DMA is in flight.
            for h in range(HPC):
                c0 = h * (HD + 1) + HD
                nc.vector.tensor_copy(
                    v_sb[:, :, c0:c0 + 1],
                    ones16_sb.rearrange('p (n o) -> p n o', o=1))

            # deferred projection work, drained inside attention loops.
            # att(0): fill with V 6..9 and kT blocks 8..11 (for att(1)).
            fillers += [lambda: emit_v(6), lambda: emit_v(7),
                        lambda: emit_qk('k', 0, 1024, 512),
                        lambda: emit_qk('k', 1, 1024, 512),
                        lambda: emit_v(8), lambda: emit_v(9)]
            emit_att(0, fill_at=[{0, 2, 4}, {0, 2, 4}], norm='chunk_gps')

            # att(1): fill with qT chunk-1, V 10..13 (for att(2))
            fillers += [lambda: emit_qk('q', 0, 1024, 512),
                        lambda: emit_qk('q', 0, 1536, 512),
                        lambda: emit_qk('q', 1, 1024, 512),
                        lambda: emit_qk('q', 1, 1536, 512),
                        lambda: emit_v(10), lambda: emit_v(11),
                        lambda: emit_v(12), lambda: emit_v(13)]
            emit_att(1, fill_at=[{0, 2, 4, 6}, {0, 2, 4, 6}],
                     norm='chunk_gps')

            # kT tail blocks 12..15 + out_proj(0) + V 14,15 during att(2)
            fillers += [lambda: emit_qk('k', 0, 1536, 512),
                        lambda: emit_qk('k', 1, 1536, 512)]
            for m in range(4):
                for n in range(2):
                    fillers.append(
                        lambda m=m, n=n: emit_oproj(m, n, 'dve'))
            fillers += [lambda: emit_v(14), lambda: emit_v(15)]
            emit_att(2, fill_at=[{0, 2, 4, 6, 8, 10},
                                 {0, 2, 4, 6, 8, 10}], norm='chunk_pe')

            # out_proj(1) and out_proj(2) during att(3)
            for m in range(4, 12):
                for n in range(2):
                    fillers.append(
                        lambda m=m, n=n: emit_oproj(m, n, 'dve'))
            emit_att(3, fill_at=[{0, 2, 4, 6, 8, 10, 12, 14},
                                 {0, 2, 4, 6, 8, 10, 12, 14}],
                     norm='pair_pe')
            drain(len(fillers))

            # tail: out_proj(3); alternate staging engines (both idle now)
            for i, (m, n) in enumerate(
                    [(m, n) for m in range(12, 16) for n in range(2)]):
                emit_oproj(m, n, 'act' if i % 2 == 0 else 'dve')

    nc.compile()
    return nc


def _get_module():
    if 'nc' not in _MODULE_CACHE:
        _MODULE_CACHE['nc'] = _build_module()
    return _MODULE_CACHE['nc']


def _make_in_maps(x, in_proj_w, in_proj_b, out_proj_w):
    import ml_dtypes
    bf = ml_dtypes.bfloat16
    x = np.asarray(x, np.float32)
    in_proj_w = np.asarray(in_proj_w, np.float32)
    in_proj_b = np.asarray(in_proj_b, np.float32)
    out_proj_w = np.asarray(out_proj_w, np.float32)

    mask01b = (np.arange(P)[:, None] <= np.arange(P)[None, :])

    xT = [np.ascontiguousarray(x[b].T) for b in range(B)]
    in_maps = []
    for core in range(NCORES):
        b, hg = core // 4, core % 4
        sl = slice(DL * hg, DL * hg + DL)
        wq = in_proj_w[0 * D:1 * D][sl]
        wk = in_proj_w[1 * D:2 * D][sl]
        wv = in_proj_w[2 * D:3 * D][sl]
        bq = in_proj_b[0 * D:1 * D][sl]
        bk = in_proj_b[1 * D:2 * D][sl]
        bv = in_proj_b[2 * D:3 * D][sl]
        in_maps.append({
            'xT': xT[b].astype(bf),
            'wqT': np.ascontiguousarray(wq.T).astype(bf),
            'wkT': np.ascontiguousarray(wk.T).astype(bf),
            'wvT': np.ascontiguousarray(wv.T).astype(bf),
            'woT': np.ascontiguousarray(out_proj_w[:, sl].T).astype(bf),
            'bq2': np.ascontiguousarray((bq * SCALE).reshape(2, P).T),
            'bk2': np.ascontiguousarray(bk.reshape(2, P).T),
            'bvrow': bv.reshape(1, DL).copy(),
            'mask01': mask01b.astype(ml_dtypes.bfloat16),
            'onesr': np.ones((P, HD), np.float32),
            'onescol': np.ones((P, 16, 1), ml_dtypes.bfloat16),
        })
    return in_maps


def kernel(x, in_proj_w, in_proj_b, out_proj_w, out_proj_b):
    global LAST_RESULTS
    _install_ntff_shim()
    from concourse import bass_utils

    nc = _get_module()
    in_maps = _make_in_maps(x, in_proj_w, in_proj_b, out_proj_w)
    res = bass_utils.run_bass_kernel_spmd(
        nc, in_maps, core_ids=list(range(NCORES)),
        trace=TRACE,
        **({'trace_cores': TRACE_CORES} if TRACE_CORES else {}))
    LAST_RESULTS = res

    out = np.zeros((B, S, D), np.float32)
    for core in range(NCORES):
        out[core // 4] += np.asarray(res.results[core]['out'], np.float32)
    out += np.asarray(out_proj_b, np.float32)
    return out
